# revision 10
# baseline (speedup 1.0000x reference)
"""Trainium2 Bass kernel: single-head causal attention (B=8, T=2048, D=1024, HS=64).

Sharding: data-parallel over batch B -- one batch element per NeuronCore (8 cores).
Host-side prep (part of sharding/layout): per-core x is passed transposed (d-major)
bf16; weights packed/transposed bf16; the output is produced transposed [HS, T]
and un-transposed on the host.

Per-core device algorithm (all matmul operands bf16, PSUM accumulation f32):
  x.T is loaded query-block-major so block-0 projections start ~3us in.
  [Q.T; K.T] stacked on partitions = [wq; wk].T-chunks @ x.T (PSUM-accumulated);
  qb is added during the Q evacuation (kb is softmax-row-invariant and dropped).
  V.T similarly; PE-transposed to natural V [tk, h]; vb folded into Vn (with the
  appended ones-column the denominator carries vb exactly).
  Attention in transposed layout per 512-query block: S.T pairs of two 128-key
  chunks share one 2-bank PSUM tile; causal masking via a PE matmul that
  accumulates an additive 0/-30000 triangle constant; exp on ACT over the whole
  1024-wide pair (diag pairs: two sliced exps); P.T chunks (bf16) feed the
  O.T accumulation one pair behind, keeping PE/ACT pipelined.
  Normalization in transposed layout: reciprocal of the ones-row, PE broadcast
  of 1/denom to 64 partitions, one DVE multiply, DMA out as out.T.
"""
import os
import sys

for _p in ("/opt/trn_rl_repo", "/root/.axon_site/_ro/trn_rl_repo"):
    if _p not in sys.path and os.path.isdir(_p):
        sys.path.append(_p)

import numpy as np
import jax

try:
    jax.config.update("jax_compilation_cache_dir", "/tmp/jax_neff_cache")
    jax.config.update("jax_persistent_cache_min_compile_time_secs", 1.0)
    jax.config.update("jax_persistent_cache_min_entry_size_bytes", -1)
except Exception:
    pass

import concourse.mybir as mybir
import concourse.tile as tile
from concourse import bacc
from concourse.bass_utils import run_bass_kernel_spmd
from concourse.masks import make_identity

B, T, D, HS = 8, 2048, 1024, 64
NCORES = 8
QB = 512            # query block (PSUM bank width f32)
KC = 128            # key chunk (partition dim of S.T tiles)
NQB = T // QB       # 4
NKC = T // KC       # 16
ND = D // 128       # 8 contraction chunks
NEG = -30000.0      # additive causal mask value (exp(scale*NEG) == 0)

MM_MODE = os.environ.get("BASS_MM_MODE", "bf16")   # "f32" | "f32r" | "bf16"
FALLBACK_MODE = "f32r"  # numerically safe mode if the fast mode misbehaves on HW

F32 = mybir.dt.float32
_MM_DTS = {"f32": F32, "f32r": mybir.dt.float32r, "bf16": mybir.dt.bfloat16}


def build(mode=None):
    MM = _MM_DTS[mode or MM_MODE]
    nc = bacc.Bacc(None)
    xT = nc.declare_dram_parameter("xT", [D, T], MM, isOutput=False)
    wqkT = nc.declare_dram_parameter("wqkT", [D, 2 * HS], MM, isOutput=False)
    wvT = nc.declare_dram_parameter("wvT", [D, HS], MM, isOutput=False)
    qb0 = nc.declare_dram_parameter("qb0", [128, 1], F32, isOutput=False)
    vbB4 = nc.declare_dram_parameter("vbB4", [128, 4 * HS], F32, isOutput=False)
    outT = nc.declare_dram_parameter("outT", [HS, T], F32, isOutput=True)

    scale = float(1.0 / np.sqrt(HS))

    with tile.TileContext(nc) as tc:
        with tc.tile_pool(name="const", bufs=1) as cpool, \
             tc.tile_pool(name="big", bufs=1) as bpool, \
             tc.tile_pool(name="vtr", bufs=2) as vpool, \
             tc.tile_pool(name="pex", bufs=6) as epool, \
             tc.tile_pool(name="rcp", bufs=2) as rpool, \
             tc.tile_pool(name="ob", bufs=2) as opool, \
             tc.tile_pool(name="psA", bufs=4, space="PSUM") as psA, \
             tc.tile_pool(name="psS", bufs=2, space="PSUM") as psS:

            # ---- DMAs first (issue cost ~0.6us per dma_start on the issuing
            # engine; spread over the two otherwise-idle engines, block-major
            # so block-0 compute starts early) ----
            xTs = bpool.tile([128, ND, T], MM, tag="xTs")
            engs = (nc.sync, nc.gpsimd)
            for j in range(NQB):
                jsl = slice(j * QB, (j + 1) * QB)
                for dc in range(ND):
                    engs[dc % 2].dma_start(
                        xTs[:, dc, jsl], xT[dc * 128:(dc + 1) * 128, jsl])
            wqk_t = cpool.tile([128, ND, 2 * HS], MM, tag="wqk")
            nc.scalar.dma_start(wqk_t[:], wqkT[:].rearrange("(c p) m -> p c m", p=128))
            wv_t = cpool.tile([128, ND, HS], MM, tag="wv")
            nc.scalar.dma_start(wv_t[:], wvT[:].rearrange("(c p) m -> p c m", p=128))
            qb_t = cpool.tile([128, 1], F32, tag="qb")
            nc.scalar.dma_start(qb_t[:], qb0[:])
            vb_t = cpool.tile([128, 4, HS], F32, tag="vb")
            nc.scalar.dma_start(vb_t[:], vbB4[:].rearrange("p (c m) -> p c m", m=HS))

            # warm the exp table on ACT while DMAs land
            dummy = cpool.tile([128, 1], F32, tag="dummy")
            nc.scalar.activation(dummy[:], qb_t[:],
                                 mybir.ActivationFunctionType.Exp)

            # ---- constants ----
            id_32 = cpool.tile([128, 128], F32, tag="id_32")
            make_identity(nc, id_32[:])
            # additive causal mask for diagonal S.T chunks:
            # cols 0:128 = triangle (0 iff f >= p else NEG), cols 128:512 = 0
            maskW_f = cpool.tile([128, QB], F32, tag="maskW_f")
            nc.gpsimd.memset(maskW_f[:], 0.0)
            nc.gpsimd.affine_select(
                out=maskW_f[:, 0:128], in_=maskW_f[:, 0:128],
                compare_op=mybir.AluOpType.is_ge,
                fill=NEG, base=0,
                pattern=[[1, 128]], channel_multiplier=-1)
            if MM is F32:
                id_mm, maskW = id_32, maskW_f
            else:
                id_mm = cpool.tile([128, 128], MM, tag="id_mm")
                nc.vector.tensor_copy(id_mm[:], id_32[:])
                maskW = cpool.tile([128, QB], MM, tag="maskW")
                nc.vector.tensor_copy(maskW[:], maskW_f[:])

            # warm the PE clock gate with throwaway transposes
            wu = psA.tile([128, 128], MM, tag="a")
            for _ in range(20):
                nc.tensor.transpose(wu[:], id_mm[:], id_mm[:])

            # persistents
            QT = bpool.tile([64, T], MM, tag="QT")
            KT = bpool.tile([64, T], MM, tag="KT")
            Vn = bpool.tile([128, NKC, HS + 1], MM, tag="Vn")
            ones16 = cpool.tile([128, NKC, 1], F32, tag="ones16")
            nc.gpsimd.memset(ones16[:], 1.0)
            nc.vector.tensor_copy(Vn[:, :, HS:HS + 1], ones16[:])

            for j in range(NQB):
                jsl = slice(j * QB, (j + 1) * QB)
                # -- QK projection --
                psqk = psA.tile([128, QB], F32, tag="a")
                for dc in range(ND):
                    nc.tensor.matmul(psqk[:], wqk_t[:, dc, :], xTs[:, dc, jsl],
                                     start=(dc == 0), stop=(dc == ND - 1))
                nc.vector.tensor_scalar_add(QT[:, jsl], psqk[0:64, :],
                                            qb_t[0:64, :])
                nc.vector.tensor_copy(KT[:, jsl], psqk[64:128, :])
                # -- V projection + naturalization --
                psv = psA.tile([128, QB], F32, tag="a")
                for dc in range(ND):
                    nc.tensor.matmul(psv[0:64, :], wv_t[:, dc, :], xTs[:, dc, jsl],
                                     start=(dc == 0), stop=(dc == ND - 1))
                vtr = vpool.tile([64, QB], MM, tag="vtr")
                nc.vector.tensor_copy(vtr[:], psv[0:64, :])
                vsc = psA.tile([128, 4, HS], MM, tag="a")
                for tt in range(4):
                    nc.tensor.transpose(vsc[:, tt, :],
                                        vtr[:, tt * 128:(tt + 1) * 128],
                                        id_mm[0:64, 0:64])
                nc.vector.scalar_tensor_tensor(
                    Vn[:, 4 * j:4 * j + 4, 0:HS], vsc[:], 1.0, vb_t[:],
                    op0=mybir.AluOpType.mult, op1=mybir.AluOpType.add)

                # -- attention for query block j --
                npair = 2 * j + 2
                po = psA.tile([128, QB], F32, tag="a")   # rows 0:HS+1 used
                pes = []

                def emit_pv(p):
                    """PV accumulation for pair p (chunks 2p, 2p+1)."""
                    pe = pes[p]
                    for k in range(2):
                        c = 2 * p + k
                        r = c - 4 * j
                        f0 = max(0, 128 * r)
                        nc.tensor.matmul(po[0:HS + 1, f0:QB], Vn[:, c, :],
                                         pe[:, k, f0:QB],
                                         start=(c == 0), stop=(c == 4 * j + 3))

                for p in range(npair):
                    ps2 = psS.tile([128, 2, QB], F32, tag="s")
                    pe = epool.tile([128, 2, QB], MM, tag="pe")
                    diag = p >= npair - 2
                    for k in range(2):
                        c = 2 * p + k
                        r = c - 4 * j
                        f0 = max(0, 128 * r)
                        qs = slice(j * QB + f0, (j + 1) * QB)
                        nc.tensor.matmul(ps2[:, k, f0:QB],
                                         KT[:, c * 128:(c + 1) * 128],
                                         QT[:, qs],
                                         start=True, stop=(r < 0))
                        if r >= 0:
                            # additive triangle mask (+zeros beyond) via PE
                            nc.tensor.matmul(ps2[:, k, f0:QB], id_mm[:],
                                             maskW[:, 0:QB - f0],
                                             start=False, stop=True)
                    if not diag:
                        nc.scalar.activation(pe[:], ps2[:],
                                             mybir.ActivationFunctionType.Exp,
                                             scale=scale)
                    else:
                        for k in range(2):
                            c = 2 * p + k
                            f0 = 128 * (c - 4 * j)
                            nc.scalar.activation(
                                pe[:, k, f0:QB], ps2[:, k, f0:QB],
                                mybir.ActivationFunctionType.Exp, scale=scale)
                    pes.append(pe)
                    if p >= 1:
                        emit_pv(p - 1)
                emit_pv(npair - 1)

                # -- normalize in transposed layout + store out.T --
                den = rpool.tile([1, QB], F32, tag="den")
                nc.vector.tensor_copy(den[:], po[HS:HS + 1, :])
                rc = rpool.tile([1, QB], F32, tag="rc")
                nc.vector.reciprocal_approx_fast(rc[:], den[:])
                rcb = rpool.tile([HS, QB], F32, tag="rcb")
                nc.gpsimd.partition_broadcast(rcb[:], rc[:], channels=HS)
                obn = opool.tile([HS, QB], F32, tag="obn")
                nc.vector.tensor_mul(obn[:], po[0:HS, :], rcb[:])
                nc.sync.dma_start(outT[:, jsl], obn[:])

    nc.compile()
    return nc


_RUNNERS = {}


def _get_runner(mode=None):
    mode = mode or MM_MODE
    if mode not in _RUNNERS:
        _RUNNERS[mode] = build(mode)
    return _RUNNERS[mode]


def _host_dt(mode=None):
    if (mode or MM_MODE) == "bf16":
        import ml_dtypes
        return ml_dtypes.bfloat16
    return np.float32


def make_in_maps(x, wq_w, wq_b, wk_w, wk_b, wv_w, wv_b, mode=None):
    hd = _host_dt(mode)
    x = np.asarray(x, np.float32)
    wqkT = np.ascontiguousarray(
        np.concatenate([np.asarray(wq_w, np.float32),
                        np.asarray(wk_w, np.float32)], axis=0).T).astype(hd)
    wvT = np.ascontiguousarray(np.asarray(wv_w, np.float32).T).astype(hd)
    qb0 = np.concatenate([np.asarray(wq_b, np.float32),
                          np.zeros(HS, np.float32)])[:, None].copy()
    vbB4 = np.ascontiguousarray(np.broadcast_to(
        np.tile(np.asarray(wv_b, np.float32), 4), (128, 4 * HS)))
    in_maps = []
    for b in range(B):
        in_maps.append({
            "xT": np.ascontiguousarray(x[b].T).astype(hd),
            "wqkT": wqkT, "wvT": wvT, "qb0": qb0, "vbB4": vbB4,
        })
    return in_maps


def run(in_maps, trace=False, tmpdir=None, mode=None):
    nc = _get_runner(mode)
    return run_bass_kernel_spmd(nc, in_maps, core_ids=list(range(NCORES)),
                                trace=trace, tmpdir=tmpdir)


def _canary_ok(out, x, wq_w, wq_b, wk_w, wk_b, wv_w, wv_b):
    """Cheap exact check of causal rows t=0,1 (closed-form, tiny host cost)."""
    x2 = np.asarray(x, np.float32)[:, 0:2, :].astype(np.float64)      # [B,2,D]
    q = x2 @ np.asarray(wq_w, np.float64).T + np.asarray(wq_b, np.float64)
    k = x2 @ np.asarray(wk_w, np.float64).T + np.asarray(wk_b, np.float64)
    v = x2 @ np.asarray(wv_w, np.float64).T + np.asarray(wv_b, np.float64)
    exp0 = v[:, 0, :]                                                 # [B,HS]
    s = np.einsum("bh,bsh->bs", q[:, 1, :], k) / np.sqrt(HS)          # [B,2]
    w = np.exp(s - s.max(-1, keepdims=True))
    w = w / w.sum(-1, keepdims=True)
    exp1 = np.einsum("bs,bsh->bh", w, v)
    got = np.stack([out[:, 0, :], out[:, 1, :]], axis=1)
    want = np.stack([exp0, exp1], axis=1)
    rel = np.abs(got - want) / max(np.abs(want).max(), 1e-6)
    return np.isfinite(got).all() and rel.max() < 3e-2


def _gather(res):
    return np.stack(
        [np.ascontiguousarray(np.asarray(res.results[b]["outT"], np.float32).T)
         for b in range(B)], axis=0)


def kernel(x, wq_w, wq_b, wk_w, wk_b, wv_w, wv_b):
    args = (x, wq_w, wq_b, wk_w, wk_b, wv_w, wv_b)
    res = run(make_in_maps(*args, mode=MM_MODE), mode=MM_MODE)
    out = _gather(res)
    if MM_MODE != FALLBACK_MODE and not _canary_ok(out, *args):
        res = run(make_in_maps(*args, mode=FALLBACK_MODE), mode=FALLBACK_MODE)
        out = _gather(res)
    return out


# revision 13
# speedup vs baseline: 1.0296x; 1.0296x over previous
"""Trainium2 Bass kernel: single-head causal attention (B=8, T=2048, D=1024, HS=64).

Sharding: data-parallel over batch B -- one batch element per NeuronCore (8 cores).
Host-side prep (part of sharding/layout): per-core x is passed transposed (d-major)
bf16; weights packed/transposed bf16; the output is produced transposed [HS, T]
and un-transposed on the host.

Per-core device algorithm (all matmul operands bf16, PSUM accumulation f32):
  x.T is loaded query-block-major so block-0 projections start ~3us in.
  [Q.T; K.T] stacked on partitions = [wq; wk].T-chunks @ x.T (PSUM-accumulated);
  qb is added during the Q evacuation (kb is softmax-row-invariant and dropped).
  V.T similarly; PE-transposed to natural V [tk, h]; vb folded into Vn (with the
  appended ones-column the denominator carries vb exactly).
  Attention in transposed layout per 512-query block: S.T pairs of two 128-key
  chunks share one 2-bank PSUM tile; causal masking via a PE matmul that
  accumulates an additive 0/-30000 triangle constant; exp on ACT over the whole
  1024-wide pair (diag pairs: two sliced exps); P.T chunks (bf16) feed the
  O.T accumulation one pair behind, keeping PE/ACT pipelined.
  Normalization in transposed layout: reciprocal of the ones-row, PE broadcast
  of 1/denom to 64 partitions, one DVE multiply, DMA out as out.T.
"""
import os
import sys

for _p in ("/opt/trn_rl_repo", "/root/.axon_site/_ro/trn_rl_repo"):
    if _p not in sys.path and os.path.isdir(_p):
        sys.path.append(_p)

import numpy as np
import jax

try:
    jax.config.update("jax_compilation_cache_dir", "/tmp/jax_neff_cache")
    jax.config.update("jax_persistent_cache_min_compile_time_secs", 1.0)
    jax.config.update("jax_persistent_cache_min_entry_size_bytes", -1)
except Exception:
    pass

import concourse.mybir as mybir
import concourse.tile as tile
from concourse import bacc
from concourse.bass_utils import run_bass_kernel_spmd
from concourse.masks import make_identity

B, T, D, HS = 8, 2048, 1024, 64
NCORES = 8
QB = 512            # query block (PSUM bank width f32)
KC = 128            # key chunk (partition dim of S.T tiles)
NQB = T // QB       # 4
NKC = T // KC       # 16
ND = D // 128       # 8 contraction chunks
NEG = -30000.0      # additive causal mask value (exp(scale*NEG) == 0)

MM_MODE = os.environ.get("BASS_MM_MODE", "bf16")   # "f32" | "f32r" | "bf16"
FALLBACK_MODE = "f32r"  # numerically safe mode if the fast mode misbehaves on HW

F32 = mybir.dt.float32
_MM_DTS = {"f32": F32, "f32r": mybir.dt.float32r, "bf16": mybir.dt.bfloat16}


def build(mode=None):
    MM = _MM_DTS[mode or MM_MODE]
    nc = bacc.Bacc(None)
    xT = nc.declare_dram_parameter("xT", [D, T], MM, isOutput=False)
    wqkT = nc.declare_dram_parameter("wqkT", [D, 2 * HS], MM, isOutput=False)
    wvT = nc.declare_dram_parameter("wvT", [D, HS], MM, isOutput=False)
    qb0 = nc.declare_dram_parameter("qb0", [128, 1], F32, isOutput=False)
    vbB4 = nc.declare_dram_parameter("vbB4", [128, 4 * HS], F32, isOutput=False)
    outT = nc.declare_dram_parameter("outT", [HS, T], F32, isOutput=True)

    scale = float(1.0 / np.sqrt(HS))

    with tile.TileContext(nc) as tc:
        with tc.tile_pool(name="const", bufs=1) as cpool, \
             tc.tile_pool(name="big", bufs=1) as bpool, \
             tc.tile_pool(name="vtr", bufs=2) as vpool, \
             tc.tile_pool(name="pex", bufs=6) as epool, \
             tc.tile_pool(name="rcp", bufs=2) as rpool, \
             tc.tile_pool(name="ob", bufs=2) as opool, \
             tc.tile_pool(name="psA", bufs=4, space="PSUM") as psA, \
             tc.tile_pool(name="psS", bufs=2, space="PSUM") as psS:

            # ---- constants first (gpsimd), so the PE warmup + projections
            # are not gated behind the DMA-issue queue ----
            id_32 = cpool.tile([128, 128], F32, tag="id_32")
            make_identity(nc, id_32[:])
            # additive causal mask for diagonal S.T chunks:
            # cols 0:128 = triangle (0 iff f >= p else NEG), cols 128:512 = 0
            maskW_f = cpool.tile([128, QB], F32, tag="maskW_f")
            nc.gpsimd.memset(maskW_f[:], 0.0)
            nc.gpsimd.affine_select(
                out=maskW_f[:, 0:128], in_=maskW_f[:, 0:128],
                compare_op=mybir.AluOpType.is_ge,
                fill=NEG, base=0,
                pattern=[[1, 128]], channel_multiplier=-1)
            if MM is F32:
                id_mm, maskW = id_32, maskW_f
            else:
                id_mm = cpool.tile([128, 128], MM, tag="id_mm")
                nc.vector.tensor_copy(id_mm[:], id_32[:])
                maskW = cpool.tile([128, QB], MM, tag="maskW")
                nc.vector.tensor_copy(maskW[:], maskW_f[:])

            # ---- DMA issues (~0.6us each on the issuing engine): block 0
            # split across sync+vector so its data is in flight earliest ----
            xTs = bpool.tile([128, ND, T], MM, tag="xTs")

            def x_dma(eng, j, dc):
                jsl = slice(j * QB, (j + 1) * QB)
                eng.dma_start(xTs[:, dc, jsl], xT[dc * 128:(dc + 1) * 128, jsl])

            wqk_t = cpool.tile([128, ND, 2 * HS], MM, tag="wqk")
            nc.scalar.dma_start(wqk_t[:], wqkT[:].rearrange("(c p) m -> p c m", p=128))
            for dc in range(0, 4):
                x_dma(nc.sync, 0, dc)
            for dc in range(4, 8):
                x_dma(nc.scalar, 0, dc)
            wv_t = cpool.tile([128, ND, HS], MM, tag="wv")
            nc.scalar.dma_start(wv_t[:], wvT[:].rearrange("(c p) m -> p c m", p=128))
            qb_t = cpool.tile([128, 1], F32, tag="qb")
            nc.scalar.dma_start(qb_t[:], qb0[:])
            vb_t = cpool.tile([128, 4, HS], F32, tag="vb")
            nc.scalar.dma_start(vb_t[:], vbB4[:].rearrange("p (c m) -> p c m", m=HS))
            for j in range(1, NQB):
                for dc in range(ND):
                    x_dma((nc.sync, nc.gpsimd)[dc % 2], j, dc)

            # warm the exp table on ACT while DMAs land
            dummy = cpool.tile([128, 1], F32, tag="dummy")
            nc.scalar.activation(dummy[:], qb_t[:],
                                 mybir.ActivationFunctionType.Exp)

            # warm the PE clock gate with throwaway transposes, bridging the
            # gap until block-0 x data lands (~40 x ~110ns cold)
            wu = psA.tile([128, 128], MM, tag="a")
            for _ in range(40):
                nc.tensor.transpose(wu[:], id_mm[:], id_mm[:])

            # persistents
            QT = bpool.tile([64, T], MM, tag="QT")
            KT = bpool.tile([64, T], MM, tag="KT")
            Vn = bpool.tile([128, NKC, HS + 1], MM, tag="Vn")
            ones16 = cpool.tile([128, NKC, 1], F32, tag="ones16")
            nc.gpsimd.memset(ones16[:], 1.0)
            nc.vector.tensor_copy(Vn[:, :, HS:HS + 1], ones16[:])

            for j in range(NQB):
                jsl = slice(j * QB, (j + 1) * QB)
                # -- QK projection --
                psqk = psA.tile([128, QB], F32, tag="a")
                for dc in range(ND):
                    nc.tensor.matmul(psqk[:], wqk_t[:, dc, :], xTs[:, dc, jsl],
                                     start=(dc == 0), stop=(dc == ND - 1))
                nc.vector.tensor_scalar_add(QT[:, jsl], psqk[0:64, :],
                                            qb_t[0:64, :])
                nc.vector.tensor_copy(KT[:, jsl], psqk[64:128, :])
                # -- V projection + naturalization --
                psv = psA.tile([128, QB], F32, tag="a")
                for dc in range(ND):
                    nc.tensor.matmul(psv[0:64, :], wv_t[:, dc, :], xTs[:, dc, jsl],
                                     start=(dc == 0), stop=(dc == ND - 1))
                vtr = vpool.tile([64, QB], MM, tag="vtr")
                nc.vector.tensor_copy(vtr[:], psv[0:64, :])
                vsc = psA.tile([128, 4, HS], MM, tag="a")
                for tt in range(4):
                    nc.tensor.transpose(vsc[:, tt, :],
                                        vtr[:, tt * 128:(tt + 1) * 128],
                                        id_mm[0:64, 0:64])
                nc.vector.scalar_tensor_tensor(
                    Vn[:, 4 * j:4 * j + 4, 0:HS], vsc[:], 1.0, vb_t[:],
                    op0=mybir.AluOpType.mult, op1=mybir.AluOpType.add)

                # -- attention for query block j --
                npair = 2 * j + 2
                po = psA.tile([128, QB], F32, tag="a")   # rows 0:HS+1 used
                pes = []

                def emit_pv(p):
                    """PV accumulation for pair p (chunks 2p, 2p+1)."""
                    pe = pes[p]
                    for k in range(2):
                        c = 2 * p + k
                        r = c - 4 * j
                        f0 = max(0, 128 * r)
                        nc.tensor.matmul(po[0:HS + 1, f0:QB], Vn[:, c, :],
                                         pe[:, k, f0:QB],
                                         start=(c == 0), stop=(c == 4 * j + 3))

                for p in range(npair):
                    ps2 = psS.tile([128, 2, QB], F32, tag="s")
                    pe = epool.tile([128, 2, QB], MM, tag="pe")
                    diag = p >= npair - 2
                    for k in range(2):
                        c = 2 * p + k
                        r = c - 4 * j
                        f0 = max(0, 128 * r)
                        qs = slice(j * QB + f0, (j + 1) * QB)
                        nc.tensor.matmul(ps2[:, k, f0:QB],
                                         KT[:, c * 128:(c + 1) * 128],
                                         QT[:, qs],
                                         start=True, stop=(r < 0))
                        if r >= 0:
                            # additive triangle mask (+zeros beyond) via PE
                            nc.tensor.matmul(ps2[:, k, f0:QB], id_mm[:],
                                             maskW[:, 0:QB - f0],
                                             start=False, stop=True)
                    if not diag:
                        nc.scalar.activation(pe[:], ps2[:],
                                             mybir.ActivationFunctionType.Exp,
                                             scale=scale)
                    else:
                        for k in range(2):
                            c = 2 * p + k
                            f0 = 128 * (c - 4 * j)
                            nc.scalar.activation(
                                pe[:, k, f0:QB], ps2[:, k, f0:QB],
                                mybir.ActivationFunctionType.Exp, scale=scale)
                    pes.append(pe)
                    if p >= 1:
                        emit_pv(p - 1)
                emit_pv(npair - 1)

                # -- normalize in transposed layout + store out.T --
                den = rpool.tile([1, QB], F32, tag="den")
                nc.vector.tensor_copy(den[:], po[HS:HS + 1, :])
                rc = rpool.tile([1, QB], F32, tag="rc")
                nc.vector.reciprocal_approx_fast(rc[:], den[:])
                rcb = rpool.tile([HS, QB], F32, tag="rcb")
                nc.gpsimd.partition_broadcast(rcb[:], rc[:], channels=HS)
                obn = opool.tile([HS, QB], F32, tag="obn")
                nc.vector.tensor_mul(obn[:], po[0:HS, :], rcb[:])
                nc.sync.dma_start(outT[:, jsl], obn[:])

    nc.compile()
    return nc


_RUNNERS = {}


def _get_runner(mode=None):
    mode = mode or MM_MODE
    if mode not in _RUNNERS:
        _RUNNERS[mode] = build(mode)
    return _RUNNERS[mode]


def _host_dt(mode=None):
    if (mode or MM_MODE) == "bf16":
        import ml_dtypes
        return ml_dtypes.bfloat16
    return np.float32


def make_in_maps(x, wq_w, wq_b, wk_w, wk_b, wv_w, wv_b, mode=None):
    hd = _host_dt(mode)
    x = np.asarray(x, np.float32)
    wqkT = np.ascontiguousarray(
        np.concatenate([np.asarray(wq_w, np.float32),
                        np.asarray(wk_w, np.float32)], axis=0).T).astype(hd)
    wvT = np.ascontiguousarray(np.asarray(wv_w, np.float32).T).astype(hd)
    qb0 = np.concatenate([np.asarray(wq_b, np.float32),
                          np.zeros(HS, np.float32)])[:, None].copy()
    vbB4 = np.ascontiguousarray(np.broadcast_to(
        np.tile(np.asarray(wv_b, np.float32), 4), (128, 4 * HS)))
    in_maps = []
    for b in range(B):
        in_maps.append({
            "xT": np.ascontiguousarray(x[b].T).astype(hd),
            "wqkT": wqkT, "wvT": wvT, "qb0": qb0, "vbB4": vbB4,
        })
    return in_maps


def run(in_maps, trace=False, tmpdir=None, mode=None):
    nc = _get_runner(mode)
    return run_bass_kernel_spmd(nc, in_maps, core_ids=list(range(NCORES)),
                                trace=trace, tmpdir=tmpdir)


def _canary_ok(out, x, wq_w, wq_b, wk_w, wk_b, wv_w, wv_b):
    """Cheap exact check of causal rows t=0,1 (closed-form, tiny host cost)."""
    x2 = np.asarray(x, np.float32)[:, 0:2, :].astype(np.float64)      # [B,2,D]
    q = x2 @ np.asarray(wq_w, np.float64).T + np.asarray(wq_b, np.float64)
    k = x2 @ np.asarray(wk_w, np.float64).T + np.asarray(wk_b, np.float64)
    v = x2 @ np.asarray(wv_w, np.float64).T + np.asarray(wv_b, np.float64)
    exp0 = v[:, 0, :]                                                 # [B,HS]
    s = np.einsum("bh,bsh->bs", q[:, 1, :], k) / np.sqrt(HS)          # [B,2]
    w = np.exp(s - s.max(-1, keepdims=True))
    w = w / w.sum(-1, keepdims=True)
    exp1 = np.einsum("bs,bsh->bh", w, v)
    got = np.stack([out[:, 0, :], out[:, 1, :]], axis=1)
    want = np.stack([exp0, exp1], axis=1)
    rel = np.abs(got - want) / max(np.abs(want).max(), 1e-6)
    return np.isfinite(got).all() and rel.max() < 3e-2


def _gather(res):
    return np.stack(
        [np.ascontiguousarray(np.asarray(res.results[b]["outT"], np.float32).T)
         for b in range(B)], axis=0)


def kernel(x, wq_w, wq_b, wk_w, wk_b, wv_w, wv_b):
    args = (x, wq_w, wq_b, wk_w, wk_b, wv_w, wv_b)
    res = run(make_in_maps(*args, mode=MM_MODE), mode=MM_MODE)
    out = _gather(res)
    if MM_MODE != FALLBACK_MODE and not _canary_ok(out, *args):
        res = run(make_in_maps(*args, mode=FALLBACK_MODE), mode=FALLBACK_MODE)
        out = _gather(res)
    return out


# revision 17
# speedup vs baseline: 1.0436x; 1.0135x over previous
"""Trainium2 Bass kernel: single-head causal attention (B=8, T=2048, D=1024, HS=64).

Sharding: data-parallel over batch B -- one batch element per NeuronCore (8 cores).
Host-side prep (part of sharding/layout): per-core x is passed transposed (d-major)
bf16; weights packed/transposed bf16; the output is produced transposed [HS, T]
and un-transposed on the host.

Per-core device algorithm (all matmul operands bf16, PSUM accumulation f32):
  x.T is loaded query-block-major so block-0 projections start ~3us in.
  [Q.T; K.T] stacked on partitions = [wq; wk].T-chunks @ x.T (PSUM-accumulated);
  qb is added during the Q evacuation (kb is softmax-row-invariant and dropped).
  V.T similarly; PE-transposed to natural V [tk, h]; vb folded into Vn (with the
  appended ones-column the denominator carries vb exactly).
  Attention in transposed layout per 512-query block: S.T pairs of two 128-key
  chunks share one 2-bank PSUM tile; causal masking via a PE matmul that
  accumulates an additive 0/-30000 triangle constant; exp on ACT over the whole
  1024-wide pair (diag pairs: two sliced exps); P.T chunks (bf16) feed the
  O.T accumulation one pair behind, keeping PE/ACT pipelined.
  Normalization in transposed layout: reciprocal of the ones-row, PE broadcast
  of 1/denom to 64 partitions, one DVE multiply, DMA out as out.T.
"""
import os
import sys

for _p in ("/opt/trn_rl_repo", "/root/.axon_site/_ro/trn_rl_repo"):
    if _p not in sys.path and os.path.isdir(_p):
        sys.path.append(_p)

import numpy as np
import jax

try:
    jax.config.update("jax_compilation_cache_dir", "/tmp/jax_neff_cache")
    jax.config.update("jax_persistent_cache_min_compile_time_secs", 1.0)
    jax.config.update("jax_persistent_cache_min_entry_size_bytes", -1)
except Exception:
    pass

import concourse.mybir as mybir
import concourse.tile as tile
from concourse import bacc
from concourse.bass_utils import run_bass_kernel_spmd
from concourse.masks import make_identity

B, T, D, HS = 8, 2048, 1024, 64
NCORES = 8
QB = 512            # query block (PSUM bank width f32)
KC = 128            # key chunk (partition dim of S.T tiles)
NQB = T // QB       # 4
NKC = T // KC       # 16
ND = D // 128       # 8 contraction chunks
NEG = -30000.0      # additive causal mask value (exp(scale*NEG) == 0)

MM_MODE = os.environ.get("BASS_MM_MODE", "bf16")   # "f32" | "f32r" | "bf16"
FALLBACK_MODE = "f32r"  # numerically safe mode if the fast mode misbehaves on HW

F32 = mybir.dt.float32
_MM_DTS = {"f32": F32, "f32r": mybir.dt.float32r, "bf16": mybir.dt.bfloat16}


def build(mode=None):
    MM = _MM_DTS[mode or MM_MODE]
    nc = bacc.Bacc(None)
    xT = nc.declare_dram_parameter("xT", [D, T], MM, isOutput=False)
    wqkP = nc.declare_dram_parameter("wqkP", [128, ND * 2 * HS], MM, isOutput=False)
    wvP = nc.declare_dram_parameter("wvP", [128, ND * HS], MM, isOutput=False)
    qb0 = nc.declare_dram_parameter("qb0", [128, 1], F32, isOutput=False)
    vbB4 = nc.declare_dram_parameter("vbB4", [128, 4 * HS], F32, isOutput=False)
    outT = nc.declare_dram_parameter("outT", [HS, T], F32, isOutput=True)

    scale = float(1.0 / np.sqrt(HS))

    with tile.TileContext(nc) as tc:
        with tc.tile_pool(name="const", bufs=1) as cpool, \
             tc.tile_pool(name="big", bufs=1) as bpool, \
             tc.tile_pool(name="vtr", bufs=2) as vpool, \
             tc.tile_pool(name="pex", bufs=6) as epool, \
             tc.tile_pool(name="rcp", bufs=2) as rpool, \
             tc.tile_pool(name="ob", bufs=2) as opool, \
             tc.tile_pool(name="psA", bufs=4, space="PSUM") as psA, \
             tc.tile_pool(name="psS", bufs=2, space="PSUM") as psS:

            # ---- constants first (gpsimd), so the PE warmup + projections
            # are not gated behind the DMA-issue queue ----
            id_32 = cpool.tile([128, 128], F32, tag="id_32")
            make_identity(nc, id_32[:])
            # additive causal mask for diagonal S.T chunks:
            # cols 0:128 = triangle (0 iff f >= p else NEG), cols 128:512 = 0
            maskW_f = cpool.tile([128, QB], F32, tag="maskW_f")
            nc.gpsimd.memset(maskW_f[:], 0.0)
            nc.gpsimd.affine_select(
                out=maskW_f[:, 0:128], in_=maskW_f[:, 0:128],
                compare_op=mybir.AluOpType.is_ge,
                fill=NEG, base=0,
                pattern=[[1, 128]], channel_multiplier=-1)
            if MM is F32:
                id_mm, maskW = id_32, maskW_f
            else:
                id_mm = cpool.tile([128, 128], MM, tag="id_mm")
                nc.vector.tensor_copy(id_mm[:], id_32[:])
                maskW = cpool.tile([128, QB], MM, tag="maskW")
                nc.vector.tensor_copy(maskW[:], maskW_f[:])

            # ---- DMA issues (~0.6us each on the issuing engine).  Blocks 0/1
            # load 512 cols wide (early start); blocks 2+3 merged 1024-wide so
            # the partition lines hit 2KB (full DMA throughput). ----
            xTs = bpool.tile([128, ND, T], MM, tag="xTs")

            def x_dma(eng, dc, c0, c1):
                eng.dma_start(xTs[:, dc, c0:c1], xT[dc * 128:(dc + 1) * 128, c0:c1])

            wqk_t = cpool.tile([128, ND, 2 * HS], MM, tag="wqk")
            nc.scalar.dma_start(wqk_t[:], wqkP[:].rearrange("p (c m) -> p c m", c=ND))
            for dc in range(0, 4):
                x_dma(nc.sync, dc, 0, QB)
            for dc in range(4, 8):
                x_dma(nc.scalar, dc, 0, QB)
            wv_t = cpool.tile([128, ND, HS], MM, tag="wv")
            nc.scalar.dma_start(wv_t[:], wvP[:].rearrange("p (c m) -> p c m", c=ND))
            qb_t = cpool.tile([128, 1], F32, tag="qb")
            nc.scalar.dma_start(qb_t[:], qb0[:])
            vb_t = cpool.tile([128, 4, HS], F32, tag="vb")
            nc.scalar.dma_start(vb_t[:], vbB4[:].rearrange("p (c m) -> p c m", m=HS))
            for dc in range(ND):
                x_dma(nc.gpsimd, dc, QB, 2 * QB)          # block 1
            for dc in range(ND):
                x_dma(nc.sync, dc, 2 * QB, 4 * QB)        # blocks 2+3 merged

            # warm the exp table on ACT while DMAs land
            dummy = cpool.tile([128, 1], F32, tag="dummy")
            nc.scalar.activation(dummy[:], qb_t[:],
                                 mybir.ActivationFunctionType.Exp)

            # warm the PE clock gate with throwaway transposes, bridging the
            # gap until block-0 x data lands (~40 x ~110ns cold)
            wu = psA.tile([128, 128], MM, tag="a")
            for _ in range(40):
                nc.tensor.transpose(wu[:], id_mm[:], id_mm[:])

            # persistents
            QT = bpool.tile([64, T], MM, tag="QT")
            KT = bpool.tile([64, T], MM, tag="KT")
            Vn = bpool.tile([128, NKC, HS + 1], MM, tag="Vn")
            ones16 = cpool.tile([128, NKC, 1], F32, tag="ones16")
            nc.gpsimd.memset(ones16[:], 1.0)
            nc.vector.tensor_copy(Vn[:, :, HS:HS + 1], ones16[:])

            for j in range(NQB):
                jsl = slice(j * QB, (j + 1) * QB)
                # -- QK projection --
                psqk = psA.tile([128, QB], F32, tag="a")
                for dc in range(ND):
                    nc.tensor.matmul(psqk[:], wqk_t[:, dc, :], xTs[:, dc, jsl],
                                     start=(dc == 0), stop=(dc == ND - 1))
                nc.vector.tensor_scalar_add(QT[:, jsl], psqk[0:64, :],
                                            qb_t[0:64, :])
                nc.vector.tensor_copy(KT[:, jsl], psqk[64:128, :])
                # -- V projection + naturalization --
                psv = psA.tile([128, QB], F32, tag="a")
                for dc in range(ND):
                    nc.tensor.matmul(psv[0:64, :], wv_t[:, dc, :], xTs[:, dc, jsl],
                                     start=(dc == 0), stop=(dc == ND - 1))
                vtr = vpool.tile([64, QB], MM, tag="vtr")
                nc.vector.tensor_copy(vtr[:], psv[0:64, :])
                vsc = psA.tile([128, 4, HS], MM, tag="a")
                for tt in range(4):
                    nc.tensor.transpose(vsc[:, tt, :],
                                        vtr[:, tt * 128:(tt + 1) * 128],
                                        id_mm[0:64, 0:64])
                nc.vector.scalar_tensor_tensor(
                    Vn[:, 4 * j:4 * j + 4, 0:HS], vsc[:], 1.0, vb_t[:],
                    op0=mybir.AluOpType.mult, op1=mybir.AluOpType.add)

                # -- attention for query block j --
                npair = 2 * j + 2
                po = psA.tile([128, QB], F32, tag="a")   # rows 0:HS+1 used
                pes = []

                def emit_pv(p):
                    """PV accumulation for pair p (chunks 2p, 2p+1)."""
                    pe = pes[p]
                    for k in range(2):
                        c = 2 * p + k
                        r = c - 4 * j
                        f0 = max(0, 128 * r)
                        nc.tensor.matmul(po[0:HS + 1, f0:QB], Vn[:, c, :],
                                         pe[:, k, f0:QB],
                                         start=(c == 0), stop=(c == 4 * j + 3))

                for p in range(npair):
                    ps2 = psS.tile([128, 2, QB], F32, tag="s")
                    pe = epool.tile([128, 2, QB], MM, tag="pe")
                    diag = p >= npair - 2
                    for k in range(2):
                        c = 2 * p + k
                        r = c - 4 * j
                        f0 = max(0, 128 * r)
                        qs = slice(j * QB + f0, (j + 1) * QB)
                        nc.tensor.matmul(ps2[:, k, f0:QB],
                                         KT[:, c * 128:(c + 1) * 128],
                                         QT[:, qs],
                                         start=True, stop=(r < 0))
                        if r >= 0:
                            # additive triangle mask (+zeros beyond) via PE
                            nc.tensor.matmul(ps2[:, k, f0:QB], id_mm[:],
                                             maskW[:, 0:QB - f0],
                                             start=False, stop=True)
                    if not diag:
                        nc.scalar.activation(pe[:], ps2[:],
                                             mybir.ActivationFunctionType.Exp,
                                             scale=scale)
                    else:
                        for k in range(2):
                            c = 2 * p + k
                            f0 = 128 * (c - 4 * j)
                            nc.scalar.activation(
                                pe[:, k, f0:QB], ps2[:, k, f0:QB],
                                mybir.ActivationFunctionType.Exp, scale=scale)
                    pes.append(pe)
                    if p >= 1:
                        emit_pv(p - 1)
                emit_pv(npair - 1)

                # -- normalize in transposed layout + store out.T --
                den = rpool.tile([1, QB], F32, tag="den")
                nc.vector.tensor_copy(den[:], po[HS:HS + 1, :])
                rc = rpool.tile([1, QB], F32, tag="rc")
                nc.vector.reciprocal_approx_fast(rc[:], den[:])
                rcb = rpool.tile([HS, QB], F32, tag="rcb")
                nc.gpsimd.partition_broadcast(rcb[:], rc[:], channels=HS)
                obn = opool.tile([HS, QB], F32, tag="obn")
                nc.vector.tensor_mul(obn[:], po[0:HS, :], rcb[:])
                nc.sync.dma_start(outT[:, jsl], obn[:])

    nc.compile()
    return nc


_RUNNERS = {}


def _get_runner(mode=None):
    mode = mode or MM_MODE
    if mode not in _RUNNERS:
        _RUNNERS[mode] = build(mode)
    return _RUNNERS[mode]


def _host_dt(mode=None):
    if (mode or MM_MODE) == "bf16":
        import ml_dtypes
        return ml_dtypes.bfloat16
    return np.float32


def make_in_maps(x, wq_w, wq_b, wk_w, wk_b, wv_w, wv_b, mode=None):
    hd = _host_dt(mode)
    x = np.asarray(x, np.float32)
    wqkT = np.concatenate([np.asarray(wq_w, np.float32),
                           np.asarray(wk_w, np.float32)], axis=0).T  # [D, 128]
    wqkP = np.ascontiguousarray(
        wqkT.reshape(ND, 128, 2 * HS).transpose(1, 0, 2).reshape(
            128, ND * 2 * HS)).astype(hd)
    wvT = np.asarray(wv_w, np.float32).T                              # [D, 64]
    wvP = np.ascontiguousarray(
        wvT.reshape(ND, 128, HS).transpose(1, 0, 2).reshape(
            128, ND * HS)).astype(hd)
    qb0 = np.concatenate([np.asarray(wq_b, np.float32),
                          np.zeros(HS, np.float32)])[:, None].copy()
    vbB4 = np.ascontiguousarray(np.broadcast_to(
        np.tile(np.asarray(wv_b, np.float32), 4), (128, 4 * HS)))
    in_maps = []
    for b in range(B):
        in_maps.append({
            "xT": np.ascontiguousarray(x[b].T).astype(hd),
            "wqkP": wqkP, "wvP": wvP, "qb0": qb0, "vbB4": vbB4,
        })
    return in_maps


def run(in_maps, trace=False, tmpdir=None, mode=None):
    nc = _get_runner(mode)
    return run_bass_kernel_spmd(nc, in_maps, core_ids=list(range(NCORES)),
                                trace=trace, tmpdir=tmpdir)


def _canary_ok(out, x, wq_w, wq_b, wk_w, wk_b, wv_w, wv_b):
    """Cheap exact check of causal rows t=0,1 (closed-form, tiny host cost)."""
    x2 = np.asarray(x, np.float32)[:, 0:2, :].astype(np.float64)      # [B,2,D]
    q = x2 @ np.asarray(wq_w, np.float64).T + np.asarray(wq_b, np.float64)
    k = x2 @ np.asarray(wk_w, np.float64).T + np.asarray(wk_b, np.float64)
    v = x2 @ np.asarray(wv_w, np.float64).T + np.asarray(wv_b, np.float64)
    exp0 = v[:, 0, :]                                                 # [B,HS]
    s = np.einsum("bh,bsh->bs", q[:, 1, :], k) / np.sqrt(HS)          # [B,2]
    w = np.exp(s - s.max(-1, keepdims=True))
    w = w / w.sum(-1, keepdims=True)
    exp1 = np.einsum("bs,bsh->bh", w, v)
    got = np.stack([out[:, 0, :], out[:, 1, :]], axis=1)
    want = np.stack([exp0, exp1], axis=1)
    rel = np.abs(got - want) / max(np.abs(want).max(), 1e-6)
    return np.isfinite(got).all() and rel.max() < 3e-2


def _gather(res):
    return np.stack(
        [np.ascontiguousarray(np.asarray(res.results[b]["outT"], np.float32).T)
         for b in range(B)], axis=0)


def kernel(x, wq_w, wq_b, wk_w, wk_b, wv_w, wv_b):
    args = (x, wq_w, wq_b, wk_w, wk_b, wv_w, wv_b)
    res = run(make_in_maps(*args, mode=MM_MODE), mode=MM_MODE)
    out = _gather(res)
    if MM_MODE != FALLBACK_MODE and not _canary_ok(out, *args):
        res = run(make_in_maps(*args, mode=FALLBACK_MODE), mode=FALLBACK_MODE)
        out = _gather(res)
    return out


# revision 21
# speedup vs baseline: 1.0923x; 1.0467x over previous
"""Trainium2 Bass kernel: single-head causal attention (B=8, T=2048, D=1024, HS=64).

Sharding: data-parallel over batch B -- one batch element per NeuronCore (8 cores).
Host-side prep (part of sharding/layout): per-core x is passed transposed (d-major)
bf16; weights packed/transposed bf16; the output is produced transposed [HS, T]
and un-transposed on the host.

Per-core device algorithm (all matmul operands bf16, PSUM accumulation f32):
  x.T is loaded query-block-major so block-0 projections start ~3us in.
  [Q.T; K.T] stacked on partitions = [wq; wk].T-chunks @ x.T (PSUM-accumulated);
  qb is added during the Q evacuation (kb is softmax-row-invariant and dropped).
  V.T similarly; PE-transposed to natural V [tk, h]; vb folded into Vn (with the
  appended ones-column the denominator carries vb exactly).
  Attention in transposed layout per 512-query block: S.T pairs of two 128-key
  chunks share one 2-bank PSUM tile; causal masking via a PE matmul that
  accumulates an additive 0/-30000 triangle constant; exp on ACT over the whole
  1024-wide pair (diag pairs: two sliced exps); P.T chunks (bf16) feed the
  O.T accumulation one pair behind, keeping PE/ACT pipelined.
  Normalization in transposed layout: reciprocal of the ones-row, PE broadcast
  of 1/denom to 64 partitions, one DVE multiply, DMA out as out.T.
"""
import os
import sys

for _p in ("/opt/trn_rl_repo", "/root/.axon_site/_ro/trn_rl_repo"):
    if _p not in sys.path and os.path.isdir(_p):
        sys.path.append(_p)

import numpy as np
import jax

try:
    jax.config.update("jax_compilation_cache_dir", "/tmp/jax_neff_cache")
    jax.config.update("jax_persistent_cache_min_compile_time_secs", 1.0)
    jax.config.update("jax_persistent_cache_min_entry_size_bytes", -1)
except Exception:
    pass

import concourse.mybir as mybir
import concourse.tile as tile
from concourse import bacc
from concourse.bass_utils import run_bass_kernel_spmd
from concourse.masks import make_identity

B, T, D, HS = 8, 2048, 1024, 64
NCORES = 8
QB = 512            # query block (PSUM bank width f32)
KC = 128            # key chunk (partition dim of S.T tiles)
NQB = T // QB       # 4
NKC = T // KC       # 16
ND = D // 128       # 8 contraction chunks
NEG = -30000.0      # additive causal mask value (exp(scale*NEG) == 0)

MM_MODE = os.environ.get("BASS_MM_MODE", "bf16")   # "f32" | "f32r" | "bf16"
FALLBACK_MODE = "f32r"  # numerically safe mode if the fast mode misbehaves on HW

F32 = mybir.dt.float32
_MM_DTS = {"f32": F32, "f32r": mybir.dt.float32r, "bf16": mybir.dt.bfloat16}


def build(mode=None):
    MM = _MM_DTS[mode or MM_MODE]
    nc = bacc.Bacc(None)
    xT = nc.declare_dram_parameter("xT", [D, T], MM, isOutput=False)
    wqkP = nc.declare_dram_parameter("wqkP", [128, ND * 2 * HS], MM, isOutput=False)
    wvP = nc.declare_dram_parameter("wvP", [128, ND * HS], MM, isOutput=False)
    qb0 = nc.declare_dram_parameter("qb0", [128, 1], F32, isOutput=False)
    vbB4 = nc.declare_dram_parameter("vbB4", [128, 4 * HS], F32, isOutput=False)
    outT = nc.declare_dram_parameter("outT", [HS, T], F32, isOutput=True)

    scale = float(1.0 / np.sqrt(HS))

    with tile.TileContext(nc) as tc:
        with tc.tile_pool(name="const", bufs=1) as cpool, \
             tc.tile_pool(name="big", bufs=1) as bpool, \
             tc.tile_pool(name="vtr", bufs=2) as vpool, \
             tc.tile_pool(name="pex", bufs=6) as epool, \
             tc.tile_pool(name="rcp", bufs=2) as rpool, \
             tc.tile_pool(name="ob", bufs=2) as opool, \
             tc.tile_pool(name="psA", bufs=4, space="PSUM") as psA, \
             tc.tile_pool(name="psS", bufs=2, space="PSUM") as psS:

            # ---- identity first (gpsimd), so the PE warmup is not gated
            # behind the DMA-issue queue; remaining consts after block-1 DMAs ----
            id_32 = cpool.tile([128, 128], F32, tag="id_32")
            make_identity(nc, id_32[:])
            if MM is F32:
                id_mm = id_32
            else:
                id_mm = cpool.tile([128, 128], MM, tag="id_mm")
                nc.vector.tensor_copy(id_mm[:], id_32[:])

            # ---- DMA issues (~0.6us each on the issuing engine).  Blocks 0/1
            # load 512 cols wide (early start); blocks 2+3 merged 1024-wide so
            # the partition lines hit 2KB (full DMA throughput). ----
            xTs = bpool.tile([128, ND, T], MM, tag="xTs")

            def x_dma(eng, dc, c0, c1):
                eng.dma_start(xTs[:, dc, c0:c1], xT[dc * 128:(dc + 1) * 128, c0:c1])

            wqk_t = cpool.tile([128, ND, 2 * HS], MM, tag="wqk")
            nc.scalar.dma_start(wqk_t[:], wqkP[:].rearrange("p (c m) -> p c m", c=ND))
            for dc in range(0, 4):
                x_dma(nc.sync, dc, 0, QB)
            for dc in range(4, 8):
                x_dma(nc.scalar, dc, 0, QB)
            wv_t = cpool.tile([128, ND, HS], MM, tag="wv")
            nc.scalar.dma_start(wv_t[:], wvP[:].rearrange("p (c m) -> p c m", c=ND))
            qb_t = cpool.tile([128, 1], F32, tag="qb")
            nc.scalar.dma_start(qb_t[:], qb0[:])
            vb_t = cpool.tile([128, 4, HS], F32, tag="vb")
            nc.scalar.dma_start(vb_t[:], vbB4[:].rearrange("p (c m) -> p c m", m=HS))
            for dc in range(ND):
                x_dma(nc.gpsimd, dc, QB, 2 * QB)          # block 1
            for dc in range(ND):
                x_dma(nc.sync, dc, 2 * QB, 4 * QB)        # blocks 2+3 merged

            # warm the exp table on ACT while DMAs land
            dummy = cpool.tile([128, 1], F32, tag="dummy")
            nc.scalar.activation(dummy[:], qb_t[:],
                                 mybir.ActivationFunctionType.Exp)

            # additive causal mask for diagonal S.T chunks (first needed at
            # ~13us, so built after the block-1 DMA issues):
            # cols 0:128 = triangle (0 iff f >= p else NEG), cols 128:512 = 0
            maskW_f = cpool.tile([128, QB], F32, tag="maskW_f")
            nc.gpsimd.memset(maskW_f[:], 0.0)
            nc.gpsimd.affine_select(
                out=maskW_f[:, 0:128], in_=maskW_f[:, 0:128],
                compare_op=mybir.AluOpType.is_ge,
                fill=NEG, base=0,
                pattern=[[1, 128]], channel_multiplier=-1)
            if MM is F32:
                maskW = maskW_f
            else:
                maskW = cpool.tile([128, QB], MM, tag="maskW")
                nc.vector.tensor_copy(maskW[:], maskW_f[:])

            # warm the PE clock gate with throwaway transposes, bridging the
            # gap until block-0 x data lands
            wu = psA.tile([128, 128], MM, tag="a")
            for _ in range(24):
                nc.tensor.transpose(wu[:], id_mm[:], id_mm[:])

            # persistents.  Vn is padded to 96 columns (32-strip alignment for
            # the PV matmul output partitions); cols 65:96 stay zero.
            VW = 96
            QT = bpool.tile([64, T], MM, tag="QT")
            KT = bpool.tile([64, T], MM, tag="KT")
            Vn = bpool.tile([128, NKC, VW], MM, tag="Vn")
            nc.vector.memzero(Vn[:])
            ones16 = cpool.tile([128, NKC, 1], F32, tag="ones16")
            nc.gpsimd.memset(ones16[:], 1.0)
            nc.vector.tensor_copy(Vn[:, :, HS:HS + 1], ones16[:])

            for j in range(NQB):
                jsl = slice(j * QB, (j + 1) * QB)
                # -- QK projection --
                psqk = psA.tile([128, QB], F32, tag="a")
                for dc in range(ND):
                    nc.tensor.matmul(psqk[:], wqk_t[:, dc, :], xTs[:, dc, jsl],
                                     start=(dc == 0), stop=(dc == ND - 1))
                nc.vector.tensor_scalar_add(QT[:, jsl], psqk[0:64, :],
                                            qb_t[0:64, :])
                nc.vector.tensor_copy(KT[:, jsl], psqk[64:128, :])
                # -- V projection + naturalization --
                psv = psA.tile([128, QB], F32, tag="a")
                for dc in range(ND):
                    nc.tensor.matmul(psv[0:64, :], wv_t[:, dc, :], xTs[:, dc, jsl],
                                     start=(dc == 0), stop=(dc == ND - 1))
                vtr = vpool.tile([64, QB], MM, tag="vtr")
                nc.vector.tensor_copy(vtr[:], psv[0:64, :])
                vsc = psA.tile([128, 4, HS], MM, tag="a")
                for tt in range(4):
                    nc.tensor.transpose(vsc[:, tt, :],
                                        vtr[:, tt * 128:(tt + 1) * 128],
                                        id_mm[0:64, 0:64])
                nc.vector.scalar_tensor_tensor(
                    Vn[:, 4 * j:4 * j + 4, 0:HS], vsc[:], 1.0, vb_t[:],
                    op0=mybir.AluOpType.mult, op1=mybir.AluOpType.add)

                # -- attention for query block j --
                npair = 2 * j + 2
                po = psA.tile([128, QB], F32, tag="a")   # rows 0:HS+1 used
                pes = []

                def emit_pv(p):
                    """PV accumulation for pair p (chunks 2p, 2p+1)."""
                    pe = pes[p]
                    for k in range(2):
                        c = 2 * p + k
                        r = c - 4 * j
                        f0 = max(0, 128 * r)
                        nc.tensor.matmul(po[0:VW, f0:QB], Vn[:, c, :],
                                         pe[:, k, f0:QB],
                                         start=(c == 0), stop=(c == 4 * j + 3))

                for p in range(npair):
                    ps2 = psS.tile([128, 2, QB], F32, tag="s")
                    pe = epool.tile([128, 2, QB], MM, tag="pe")
                    diag = p >= npair - 2
                    for k in range(2):
                        c = 2 * p + k
                        r = c - 4 * j
                        f0 = max(0, 128 * r)
                        qs = slice(j * QB + f0, (j + 1) * QB)
                        nc.tensor.matmul(ps2[:, k, f0:QB],
                                         KT[:, c * 128:(c + 1) * 128],
                                         QT[:, qs],
                                         start=True, stop=(r < 0))
                        if r >= 0:
                            # additive triangle mask (+zeros beyond) via PE
                            nc.tensor.matmul(ps2[:, k, f0:QB], id_mm[:],
                                             maskW[:, 0:QB - f0],
                                             start=False, stop=True)
                    if not diag:
                        nc.scalar.activation(pe[:], ps2[:],
                                             mybir.ActivationFunctionType.Exp,
                                             scale=scale)
                    else:
                        for k in range(2):
                            c = 2 * p + k
                            f0 = 128 * (c - 4 * j)
                            nc.scalar.activation(
                                pe[:, k, f0:QB], ps2[:, k, f0:QB],
                                mybir.ActivationFunctionType.Exp, scale=scale)
                    pes.append(pe)
                    if p >= 1:
                        emit_pv(p - 1)
                emit_pv(npair - 1)

                # -- normalize in transposed layout + store out.T --
                den = rpool.tile([1, QB], F32, tag="den")
                nc.vector.tensor_copy(den[:], po[HS:HS + 1, :])
                rc = rpool.tile([1, QB], F32, tag="rc")
                nc.vector.reciprocal_approx_fast(rc[:], den[:])
                rcb = rpool.tile([HS, QB], F32, tag="rcb")
                nc.gpsimd.partition_broadcast(rcb[:], rc[:], channels=HS)
                obn = opool.tile([HS, QB], F32, tag="obn")
                nc.vector.tensor_mul(obn[:], po[0:HS, :], rcb[:])
                nc.sync.dma_start(outT[:, jsl], obn[:])

    nc.compile()
    return nc


_RUNNERS = {}


def _get_runner(mode=None):
    mode = mode or MM_MODE
    if mode not in _RUNNERS:
        _RUNNERS[mode] = build(mode)
    return _RUNNERS[mode]


def _host_dt(mode=None):
    if (mode or MM_MODE) == "bf16":
        import ml_dtypes
        return ml_dtypes.bfloat16
    return np.float32


def make_in_maps(x, wq_w, wq_b, wk_w, wk_b, wv_w, wv_b, mode=None):
    hd = _host_dt(mode)
    x = np.asarray(x, np.float32)
    wqkT = np.concatenate([np.asarray(wq_w, np.float32),
                           np.asarray(wk_w, np.float32)], axis=0).T  # [D, 128]
    wqkP = np.ascontiguousarray(
        wqkT.reshape(ND, 128, 2 * HS).transpose(1, 0, 2).reshape(
            128, ND * 2 * HS)).astype(hd)
    wvT = np.asarray(wv_w, np.float32).T                              # [D, 64]
    wvP = np.ascontiguousarray(
        wvT.reshape(ND, 128, HS).transpose(1, 0, 2).reshape(
            128, ND * HS)).astype(hd)
    qb0 = np.concatenate([np.asarray(wq_b, np.float32),
                          np.zeros(HS, np.float32)])[:, None].copy()
    vbB4 = np.ascontiguousarray(np.broadcast_to(
        np.tile(np.asarray(wv_b, np.float32), 4), (128, 4 * HS)))
    in_maps = []
    for b in range(B):
        in_maps.append({
            "xT": np.ascontiguousarray(x[b].T).astype(hd),
            "wqkP": wqkP, "wvP": wvP, "qb0": qb0, "vbB4": vbB4,
        })
    return in_maps


def run(in_maps, trace=False, tmpdir=None, mode=None):
    nc = _get_runner(mode)
    return run_bass_kernel_spmd(nc, in_maps, core_ids=list(range(NCORES)),
                                trace=trace, tmpdir=tmpdir)


def _canary_ok(out, x, wq_w, wq_b, wk_w, wk_b, wv_w, wv_b):
    """Cheap exact check of causal rows t=0,1 (closed-form, tiny host cost)."""
    x2 = np.asarray(x, np.float32)[:, 0:2, :].astype(np.float64)      # [B,2,D]
    q = x2 @ np.asarray(wq_w, np.float64).T + np.asarray(wq_b, np.float64)
    k = x2 @ np.asarray(wk_w, np.float64).T + np.asarray(wk_b, np.float64)
    v = x2 @ np.asarray(wv_w, np.float64).T + np.asarray(wv_b, np.float64)
    exp0 = v[:, 0, :]                                                 # [B,HS]
    s = np.einsum("bh,bsh->bs", q[:, 1, :], k) / np.sqrt(HS)          # [B,2]
    w = np.exp(s - s.max(-1, keepdims=True))
    w = w / w.sum(-1, keepdims=True)
    exp1 = np.einsum("bs,bsh->bh", w, v)
    got = np.stack([out[:, 0, :], out[:, 1, :]], axis=1)
    want = np.stack([exp0, exp1], axis=1)
    rel = np.abs(got - want) / max(np.abs(want).max(), 1e-6)
    return np.isfinite(got).all() and rel.max() < 3e-2


def _gather(res):
    return np.stack(
        [np.ascontiguousarray(np.asarray(res.results[b]["outT"], np.float32).T)
         for b in range(B)], axis=0)


def kernel(x, wq_w, wq_b, wk_w, wk_b, wv_w, wv_b):
    args = (x, wq_w, wq_b, wk_w, wk_b, wv_w, wv_b)
    res = run(make_in_maps(*args, mode=MM_MODE), mode=MM_MODE)
    out = _gather(res)
    if MM_MODE != FALLBACK_MODE and not _canary_ok(out, *args):
        res = run(make_in_maps(*args, mode=FALLBACK_MODE), mode=FALLBACK_MODE)
        out = _gather(res)
    return out
